# revision 2
# baseline (speedup 1.0000x reference)
"""Trainium2 Bass kernel for nn_BiMambaBlock — bf16 rewrite.

Strategy (8 NeuronCores, no cross-core communication):
  - Shard (batch=4) x (sequence halves=2) -> 8 cores; 256-token halo on
    second-half cores (SSM state decays to ~0 within the halo).  Host
    reverses time and converts to bf16; kernel runs a forward scan.
  - Per core: 9 tiles of 256 tokens through
    in_proj -> causal conv -> SSD chunked scan (Q=128) -> gated RMSNorm
    -> out_proj -> LayerNorm -> MLP, all in SBUF, all matmuls bf16.
  - Decay masks are built ON-CHIP: dA = exp(dt*A) rows are broadcast
    across partitions with rank-1 PE matmuls, then a DVE scan
    (y_i = y_{i-1}*dA_i + I[j,i]) produces the per-head mask directly
    (upper-triangular by construction).  No DRAM mask traffic.
  - LayerNorm mean is folded into out_proj by centering its columns on
    host; rsqrt is computed as exp(-0.5*ln(v)) so ACT needs only the
    {exp,ln} and {silu} table sets.  ACT work is emitted in two phases
    per tile (exp-set then silu-set) to minimize table reloads.
"""

import numpy as np

# ---- dims ----
DM = 512          # d_model
DST = 64          # d_state
DI = 1024         # d_inner
NH = 16           # heads
HD = 64           # head dim
CD = 1152         # conv dim = DI + 2*DST
B, L = 4, 4096
EPS = 1e-5
HALO, SEG = 256, 2048
TOK = 256         # tokens per pipeline tile
NT = (HALO + SEG) // TOK   # 9
Q = 128           # SSD chunk
NCQ = TOK // Q    # chunks per tile

_BUILT = None
REPEAT = 1


def _patch_concourse(tile_mod, bass_mod):
    """This container's walrus accepts a single sync-wait per instruction.
    Split extra waits onto NoOp / extra Drain instructions."""
    from concourse.vector_clock import ScopedClock
    import json

    def _drain_and_barrier(self, tick_clock, wait_clock):
        nc = self.nc
        drain_inst = nc.sync.drain()
        wait_clock.add_sem_waits(drain_inst.ins,
                                 ScopedClock({None: tick_clock.global_clock}))
        si = drain_inst.ins.sync_info
        waits = list(si.on_wait) if (si is not None and si.on_wait) else []
        if len(waits) > 1:
            si.on_wait = waits[:1]
            name2h = {h.name: h for h in self.sems.allocated().values()}
            for w in waits[1:]:
                d2 = nc.sync.drain()
                d2.wait_op(name2h[w.ant_name], w.wait_value, "sem-ge")
        nc.all_engine_barrier()
        popped = nc._tile_sem_poison_stack.pop()
        assert popped is self._sem_poison
        nc.clear_and_free_semaphores(list(self.sems.allocated().values()))
        nc.all_engine_barrier()

    tile_mod.TileContext._drain_and_barrier = _drain_and_barrier

    def _split_waits(m):
        n = 0
        for f in m.get("functions", []):
            for bb in f.get("blocks", []):
                out = []
                for ins in bb.get("instructions", []):
                    si = ins.get("sync_info")
                    waits = (si or {}).get("on_wait") or []
                    if len(waits) > 1:
                        for i, w in enumerate(waits[:-1]):
                            out.append({
                                "debug": ins.get("debug", 0),
                                "engine": ins["engine"],
                                "ins": [], "outs": [],
                                "name": f"{ins['name']}-ws{i}",
                                "opcode": "NoOp",
                                "sync_info": {"on_update": [], "on_wait": [w]},
                            })
                        si["on_wait"] = waits[-1:]
                        n += 1
                    out.append(ins)
                bb["instructions"] = out
        return n

    if not getattr(bass_mod.Bass, "_wait_split_patched", False):
        orig = bass_mod.Bass.to_json_bytes

        def to_json_bytes(self):
            raw = orig(self)
            m = json.loads(raw)
            if _split_waits(m):
                raw = json.dumps(m).encode()
            return raw

        bass_mod.Bass.to_json_bytes = to_json_bytes
        bass_mod.Bass._wait_split_patched = True


def _build():
    global _BUILT
    if _BUILT is not None:
        return _BUILT
    import concourse.bass as bass
    import concourse.tile as tile
    from concourse import mybir
    from concourse.masks import make_identity
    from contextlib import ExitStack

    _patch_concourse(tile, bass)

    f32 = mybir.dt.float32
    bf16 = mybir.dt.bfloat16
    AL = mybir.AluOpType
    AF = mybir.ActivationFunctionType

    nc = bass.Bass()

    # ---- DRAM I/O (per-core) ----
    # x: bf16, feature-major, (4 k-slices of d_model) x 128 x (HALO+SEG)
    xT = nc.dram_tensor("xT", (4, 128, HALO + SEG), bf16, kind="ExternalInput")
    wz = nc.dram_tensor("wz", (4, 128, DI), bf16, kind="ExternalInput")
    wxbc = nc.dram_tensor("wxbc", (4, 128, CD), bf16, kind="ExternalInput")
    wdt = nc.dram_tensor("wdt", (4, 128, NH), bf16, kind="ExternalInput")
    wout = nc.dram_tensor("wout", (8, 128, DM), bf16, kind="ExternalInput")
    w1 = nc.dram_tensor("w1", (4, 128, DI), bf16, kind="ExternalInput")
    w2 = nc.dram_tensor("w2", (8, 128, DM), bf16, kind="ExternalInput")
    convw = nc.dram_tensor("convw", (128, 9, 4), bf16, kind="ExternalInput")
    convb = nc.dram_tensor("convb", (128, 9, 1), f32, kind="ExternalInput")
    dtb = nc.dram_tensor("dtb", (NH, 1), f32, kind="ExternalInput")
    Ah = nc.dram_tensor("Ah", (NH, 1), f32, kind="ExternalInput")
    Drep = nc.dram_tensor("Drep", (1, DI), f32, kind="ExternalInput")
    b1r = nc.dram_tensor("b1r", (1, DI), bf16, kind="ExternalInput")
    b2r = nc.dram_tensor("b2r", (1, DM), bf16, kind="ExternalInput")
    outT = nc.dram_tensor("outT", (4, 128, SEG), bf16, kind="ExternalOutput")

    with tile.TileContext(nc) as tc, ExitStack() as ctx:
        wp = ctx.enter_context(tc.tile_pool(name="wp", bufs=1))
        xp = ctx.enter_context(tc.tile_pool(name="xp", bufs=2))     # x tiles
        cq = ctx.enter_context(tc.tile_pool(name="cq", bufs=2))     # xbc/conv
        sp = ctx.enter_context(tc.tile_pool(name="sp", bufs=2))     # small
        mp = ctx.enter_context(tc.tile_pool(name="mp", bufs=2))     # masks
        zp = ctx.enter_context(tc.tile_pool(name="zp", bufs=2))     # per-chunk big
        yp = ctx.enter_context(tc.tile_pool(name="yp", bufs=2))     # Yg etc
        op = ctx.enter_context(tc.tile_pool(name="op", bufs=2))     # out stage
        st = ctx.enter_context(tc.tile_pool(name="st", bufs=2))     # state/stash
        dp = ctx.enter_context(tc.tile_pool(name="dp", bufs=2, space="DRAM"))
        pbig = ctx.enter_context(tc.tile_pool(name="pbig", bufs=2, space="PSUM"))
        ptr = ctx.enter_context(tc.tile_pool(name="ptr", bufs=2, space="PSUM"))
        pya = ctx.enter_context(tc.tile_pool(name="pya", bufs=1, space="PSUM"))
        pyb = ctx.enter_context(tc.tile_pool(name="pyb", bufs=1, space="PSUM"))

        # ---- load weights / constants ----
        def ld(name, dram, shape, dt=bf16):
            t = wp.tile(list(shape), dt, tag=name)
            nc.sync.dma_start(out=t[:], in_=dram[:])
            return t

        t_wz = [ld(f"wz{k}", wz[k], (128, DI)) for k in range(4)]
        t_wxbc = [ld(f"wxbc{k}", wxbc[k], (128, CD)) for k in range(4)]
        t_wdt = [ld(f"wdt{k}", wdt[k], (128, NH)) for k in range(4)]
        t_wout = [ld(f"wout{k}", wout[k], (128, DM)) for k in range(8)]
        t_w1 = [ld(f"w1{k}", w1[k], (128, DI)) for k in range(4)]
        t_w2 = [ld(f"w2{k}", w2[k], (128, DM)) for k in range(8)]
        t_convw = ld("convw", convw, (128, 9, 4), bf16)
        t_convb = ld("convb", convb, (128, 9, 1), f32)
        t_dtb = ld("dtb", dtb, (NH, 1), f32)
        t_A = ld("Ah", Ah, (NH, 1), f32)
        t_Dbc = wp.tile([128, DI], f32, tag="Dbc")
        nc.sync.dma_start(out=t_Dbc[:], in_=Drep[:].to_broadcast((128, DI)))

        identb = wp.tile([128, 128], bf16, tag="identb")
        make_identity(nc, identb[:])
        identf = wp.tile([128, 128], f32, tag="identf")
        make_identity(nc, identf[:])
        ones_b = wp.tile([1, TOK], bf16, tag="ones_b")
        nc.vector.memset(ones_b[:], 1.0)
        t_b1r = wp.tile([1, DI], bf16, tag="b1r")
        nc.sync.dma_start(out=t_b1r[:], in_=b1r[:])
        t_b2r = wp.tile([1, DM], bf16, tag="b2r")
        nc.sync.dma_start(out=t_b2r[:], in_=b2r[:])
        epsc = wp.tile([128, 1], f32, tag="epsc")
        nc.vector.memset(epsc[:], EPS)
        zrow = wp.tile([NH, 1], f32, tag="zrow")
        nc.vector.memset(zrow[:], 0.0)
        ones_col = wp.tile([128, 1], bf16, tag="ones_col")
        nc.vector.memset(ones_col[:], 1.0)
        # shifted-identity injection pattern for 4-head batched mask scans
        identx = wp.tile([128, 4 * (Q + 1)], bf16, tag="identx")
        nc.vector.memset(identx[:], 0.0)
        for g4 in range(4):
            nc.vector.tensor_copy(
                identx[:, g4 * (Q + 1) + 1:(g4 + 1) * (Q + 1)], identb[:])
        # persistent DRAM scratch for dA partition-broadcast (129-pitch,
        # separator column zeroed once) and for elast rows
        zsep = wp.tile([NH, Q + 1], bf16, tag="zsep")
        nc.vector.memset(zsep[:], 0.0)
        dAd = dp.tile([2, 2, NH * (Q + 1)], bf16, tag="dAd")
        for t2 in range(2):
            for c2 in range(2):
                nc.sync.dma_start(
                    out=dAd[t2][c2:c2 + 1, :].rearrange(
                        "o (h i) -> (o h) i", h=NH), in_=zsep[:])
        eld = dp.tile([2, 2, NH], bf16, tag="eld")

        # D-scaled identities: ypsA[:,h,:] += (D_h I) @ xh_h  (skip term)
        identD = wp.tile([128, NH, 128], bf16, tag="identD")
        for h4 in range(NH):
            nc.vector.tensor_scalar(identD[:, h4, :], identb[:],
                                    t_Dbc[:, h4 * HD:h4 * HD + 1], None,
                                    op0=AL.mult)
        # persistent state / conv stash
        state = st.tile([64, DI], bf16, tag="state")
        nc.vector.memset(state[:], 0.0)
        stash = st.tile([128, 9, 3], bf16, tag="stash")
        nc.vector.memset(stash[:], 0.0)

        # cross-tile staging (tile it computes Yg/ss; out_proj..MLP of
        # tile it runs during iteration it+1, after rstd in exp-phase)
        prev = None   # dict with Yg[c], ss[c] from previous tile

        for _rep in range(REPEAT):
            for it in range(NT + 1):
                first, last = (it == 0), (it == NT)
                t0 = it * TOK

                # =========================================================
                # PHASE A (exp/ln table set on ACT):
                #   dt path for tile `it` + rstd/out_proj/LN for tile it-1
                # =========================================================
                if not last:
                    # ---- x tile (feature-major bf16) ----
                    x_fm = xp.tile([128, 4, TOK], bf16, tag="x_fm")
                    for k in range(4):
                        nc.sync.dma_start(out=x_fm[:, k, :],
                                          in_=xT[k][:, t0:t0 + TOK])

                    # ---- dt path: psd = wdt.T @ x ----
                    psd = pbig.tile([NH, TOK], f32, tag="ps")
                    for k in range(4):
                        nc.tensor.matmul(psd[:], t_wdt[k][:], x_fm[:, k, :],
                                         start=(k == 0), stop=(k == 3))
                    # softplus(x+b) = ln(1 + exp(x+b))
                    dt_e = sp.tile([NH, TOK], f32, tag="dt_e")
                    nc.scalar.activation(dt_e[:], psd[:], AF.Exp,
                                         bias=t_dtb[:], scale=1.0)
                    nc.vector.tensor_scalar_add(dt_e[:], dt_e[:], 1.0)
                    dt_fm = sp.tile([NH, TOK], f32, tag="dt_fm")
                    nc.scalar.activation(dt_fm[:], dt_e[:], AF.Ln)
                    dtA_fm = sp.tile([NH, TOK], f32, tag="dtA_fm")
                    nc.vector.tensor_scalar_mul(dtA_fm[:], dt_fm[:], t_A[:])
                    dtb16 = sp.tile([NH, TOK], bf16, tag="dtb16")
                    nc.vector.tensor_copy(dtb16[:], dt_fm[:])
                    # dA = exp(dtA)  (bf16 factors for the mask scans)
                    dA_fm = sp.tile([NH, TOK], bf16, tag="dA_fm")
                    nc.scalar.activation(dA_fm[:], dtA_fm[:], AF.Exp)
                    t2 = it % 2

                    # per-chunk dt-derived quantities
                    ch = []
                    for c in range(NCQ):
                        csl = slice(c * Q, (c + 1) * Q)
                        d = {}
                        s_fm = sp.tile([NH, Q], f32, tag=f"s_fm{c}")
                        nc.vector.tensor_tensor_scan(
                            s_fm[:], dtA_fm[:, csl],
                            zrow[:].to_broadcast((NH, Q)),
                            0.0, op0=AL.add, op1=AL.add)
                        d["s_fm"] = s_fm
                        # wdec_fm = exp(s_last - s)  (per-partition scalar)
                        wde = sp.tile([NH, Q], f32, tag=f"wde{c}")
                        nc.vector.tensor_scalar(
                            wde[:], s_fm[:], s_fm[:, Q - 1:Q], None,
                            op0=AL.subtract)
                        wdec_fm = sp.tile([NH, Q], bf16, tag=f"wdecf{c}")
                        nc.scalar.activation(wdec_fm[:], wde[:], AF.Exp,
                                             scale=-1.0)
                        # E_fm = exp(s)
                        E_fm = sp.tile([NH, Q], bf16, tag=f"Ef{c}")
                        nc.scalar.activation(E_fm[:], s_fm[:], AF.Exp)
                        # elast column = exp(s_last) (bf16 [NH,1])
                        ecol = sp.tile([NH, 1], bf16, tag=f"ecol{c}")
                        nc.scalar.activation(ecol[:], s_fm[:, Q - 1:Q], AF.Exp)
                        # transposes (token-major): dt, wdec, E
                        ptx = ptr.tile([Q, 3, NH], bf16, tag="ps")
                        nc.tensor.transpose(ptx[:, 0, :], dtb16[:, csl],
                                            identb[0:NH, 0:NH])
                        nc.tensor.transpose(ptx[:, 1, :], wdec_fm[:],
                                            identb[0:NH, 0:NH])
                        nc.tensor.transpose(ptx[:, 2, :], E_fm[:],
                                            identb[0:NH, 0:NH])
                        tws = sp.tile([Q, 3, NH], bf16, tag=f"tws{c}")
                        nc.vector.tensor_copy(tws[:], ptx[:])
                        dt_tm = tws[:, 0, :]
                        wdec_tm = tws[:, 1, :]
                        E_tm = tws[:, 2, :]
                        # elast broadcast via DRAM bounce [64, 16]
                        nc.sync.dma_start(out=eld[t2][c:c + 1, :]
                                          .rearrange("o h -> (o h)"), in_=ecol[:])
                        elast_bc = sp.tile([DST, NH], bf16, tag="elbc")
                        nc.sync.dma_start(
                            out=elast_bc[:],
                            in_=eld[t2][c:c + 1, :].to_broadcast((DST, NH)))
                        d["dt_tm"], d["wdec_tm"], d["E_tm"] = dt_tm, wdec_tm, E_tm
                        d["elast_bc"] = elast_bc
                        # masks: DRAM partition-broadcast of dA, then 4
                        # batched 4-head scans (product recurrence)
                        if not first:
                            nc.sync.dma_start(
                                out=dAd[t2][c:c + 1, :].rearrange(
                                    "o (h i) -> (o h) i", h=NH)[:, 1:Q + 1],
                                in_=dA_fm[:, csl])
                            dA_bc = mp.tile([128, NH * (Q + 1)], bf16, tag="dabc")
                            nc.sync.dma_start(
                                out=dA_bc[:],
                                in_=dAd[t2][c:c + 1, :].to_broadcast(
                                    (128, NH * (Q + 1))))
                            mk = mp.tile([128, NH, Q + 1], bf16, tag="mk")
                            mkf = mk[:].rearrange("p h i -> p (h i)")
                            for g in range(4):
                                gsl = slice(g * 4 * (Q + 1), (g + 1) * 4 * (Q + 1))
                                nc.vector.tensor_tensor_scan(
                                    mkf[:, gsl], dA_bc[:, gsl], identx[:],
                                    0.0, op0=AL.mult, op1=AL.add)
                            d["mk"] = mk
                        ch.append(d)

                if prev is not None:
                    # ---- deferred tail of tile it-1 (exp/ln set) ----
                    # RMS rstd per chunk: rstd = exp(-0.5*ln(ms))
                    yn_fm = yp.tile([128, 8, TOK], bf16, tag="yn_fm")
                    for c in range(NCQ):
                        csl = slice(c * Q, (c + 1) * Q)
                        ssl = prev["ss"][c]
                        lnm = sp.tile([Q, 1], f32, tag="lnm")
                        nc.scalar.activation(lnm[:], ssl[:], AF.Ln,
                                             bias=epsc[0:Q, :], scale=1.0 / DI)
                        rstd = sp.tile([Q, 1], f32, tag="rstd")
                        nc.scalar.activation(rstd[:], lnm[:], AF.Exp, scale=-0.5)
                        yn = yp.tile([Q, DI], bf16, tag="yn")
                        nc.vector.tensor_scalar_mul(yn[:], prev["Yg"][c][:],
                                                    rstd[:])
                        # transpose yn -> feature-major (batched drains)
                        for g4 in range(2):
                            ptn = ptr.tile([128, 4, 128], bf16, tag="ps")
                            for mm in range(4):
                                nc.tensor.transpose(
                                    ptn[:, mm, :],
                                    yn[:, (g4 * 4 + mm) * 128:(g4 * 4 + mm + 1) * 128],
                                    identb[:])
                            nc.vector.tensor_copy(
                                yn_fm[:, g4 * 4:(g4 + 1) * 4, csl], ptn[:])
                    # out_proj (centered weights) -> ym psum [128, 4, TOK]
                    pym = pya.tile([128, 4, TOK], f32, tag="pya")
                    for mt in range(4):
                        for k in range(8):
                            nc.tensor.matmul(
                                pym[:, mt, :],
                                t_wout[k][:, mt * 128:(mt + 1) * 128],
                                yn_fm[:, k, :], start=(k == 0), stop=(k == 7))
                    # var = mean(ym^2): square on ACT (any set), PE reduce
                    sq4 = op.tile([128, 4, TOK], bf16, tag="sq4")
                    nc.scalar.activation(sq4[:], pym[:], AF.Square)
                    pv = ptr.tile([1, TOK], f32, tag="ps")
                    for k in range(4):
                        nc.tensor.matmul(pv[:], ones_col[:], sq4[:, k, :],
                                         start=(k == 0), stop=(k == 3))
                    lnv = sp.tile([1, TOK], f32, tag="lnv")
                    nc.scalar.activation(lnv[:], pv[:], AF.Ln,
                                         bias=epsc[0:1, :], scale=1.0 / DM)
                    rstdr = sp.tile([1, TOK], bf16, tag="rstdr")
                    nc.scalar.activation(rstdr[:], lnv[:], AF.Exp, scale=-0.5)
                    # broadcast rstd row across partitions (rank-1)
                    prs = ptr.tile([128, TOK], f32, tag="ps")
                    nc.tensor.matmul(prs[:], ones_b[:, 0:128], rstdr[:],
                                     start=True, stop=True)
                    rstd_bc = op.tile([128, TOK], bf16, tag="rstd_bc")
                    nc.vector.tensor_copy(rstd_bc[:], prs[:])
                    # ln = ym * rstd_bc  (mean already removed by centering)
                    ln_fm = op.tile([128, 4, TOK], bf16, tag="ln_fm")
                    nc.vector.tensor_tensor(
                        out=ln_fm[:], in0=pym[:],
                        in1=rstd_bc[:].rearrange("p (o t) -> p o t", o=1)
                        .to_broadcast((128, 4, TOK)), op=AL.mult)
                    prev["ln_fm"] = ln_fm

                # =========================================================
                # PHASE B (silu table set on ACT):
                #   conv/SSD/gate for tile `it` + MLP of tile it-1
                # =========================================================
                if not last:
                    # ---- in_proj xBC (feature-major) ----
                    xbc_ext = cq.tile([128, 9, 3 + TOK], bf16, tag="xbc_ext")
                    nc.vector.tensor_copy(xbc_ext[:, :, 0:3], stash[:])
                    for mp2 in range(5):
                        nmt = 2 if mp2 < 4 else 1
                        ps = pbig.tile([128, 2, TOK], f32, tag="ps")
                        for sub in range(nmt):
                            mt = mp2 * 2 + sub
                            for k in range(4):
                                nc.tensor.matmul(
                                    ps[:, sub, :],
                                    t_wxbc[k][:, mt * 128:(mt + 1) * 128],
                                    x_fm[:, k, :], start=(k == 0), stop=(k == 3))
                        nc.scalar.copy(
                            xbc_ext[:, mp2 * 2:mp2 * 2 + nmt, 3:3 + TOK],
                            ps[:, 0:nmt, :])
                    stash_new = st.tile([128, 9, 3], bf16, tag="stash")
                    nc.vector.tensor_copy(stash_new[:],
                                          xbc_ext[:, :, TOK:TOK + 3])
                    stash = stash_new

                    # ---- conv (no silu yet) ----
                    conv = cq.tile([128, 9, TOK], bf16, tag="conv")
                    for mt in range(9):
                        eng = nc.vector
                        eng.scalar_tensor_tensor(
                            conv[:, mt, :], xbc_ext[:, mt, 3:3 + TOK],
                            t_convw[:, mt, 3:4],
                            t_convb[:, mt, :].to_broadcast((128, TOK)),
                            op0=AL.mult, op1=AL.add)
                        for k in (2, 1, 0):
                            eng.scalar_tensor_tensor(
                                conv[:, mt, :], xbc_ext[:, mt, k:k + TOK],
                                t_convw[:, mt, k:k + 1],
                                conv[:, mt, :], op0=AL.mult, op1=AL.add)
                    # B/C rows: silu in feature-major
                    bc_fm = cq.tile([128, TOK], bf16, tag="bc_fm")
                    nc.scalar.activation(bc_fm[:], conv[:, 8, :], AF.Silu)

                    # ---- z (token-major, silu) ----
                    silu_z = [None, None]
                    if not first:
                        for c in range(NCQ):
                            csl = slice(c * 128, (c + 1) * 128)
                            zt = zp.tile([128, DI], bf16, tag=f"silu_z{c}")
                            for h2 in range(2):
                                ps = pbig.tile([128, 512], f32, tag="ps")
                                for k in range(4):
                                    nc.tensor.matmul(
                                        ps[:], x_fm[:, k, csl],
                                        t_wz[k][:, h2 * 512:(h2 + 1) * 512],
                                        start=(k == 0), stop=(k == 3))
                                nc.scalar.activation(
                                    zt[:, h2 * 512:(h2 + 1) * 512], ps[:], AF.Silu)
                            silu_z[c] = zt

                    # ---- per-chunk SSD ----
                    Yg_list, ss_list = [], []
                    for c in range(NCQ):
                        csl = slice(c * 128, (c + 1) * 128)
                        d = ch[c]
                        # xh: transpose (4 per psum bank) + batched silu
                        xh_tm = zp.tile([128, DI], bf16, tag="xh_tm")
                        for g4 in range(2):
                            pt = ptr.tile([128, 4, 128], bf16, tag="ps")
                            for mm in range(4):
                                nc.tensor.transpose(
                                    pt[:, mm, :],
                                    conv[:, g4 * 4 + mm, csl], identb[:])
                            nc.scalar.activation(
                                xh_tm[:, g4 * 512:(g4 + 1) * 512],
                                pt[:].rearrange("p m t -> p (m t)"), AF.Silu)
                        # B_fm / C: B rows 0-63 of bc_fm, C rows 64-127 -> move C
                        B_fm = bc_fm[0:DST, csl]
                        C_t = sp.tile([DST, Q], bf16, tag=f"C_t{c}")
                        nc.sync.dma_start(out=C_t[:], in_=bc_fm[DST:128, csl])
                        # B token-major (for state update lhsT)
                        pbt = ptr.tile([Q, DST], bf16, tag="ps")
                        nc.tensor.transpose(pbt[:], B_fm, identb[0:DST, 0:DST])
                        B_tm = sp.tile([Q, DST], bf16, tag=f"B_tm{c}")
                        nc.vector.tensor_copy(B_tm[:], pbt[:])
                        # Xp3 = dt * xh ; Xpd = Xp3 * wdec
                        Xp3 = zp.tile([128, DI], bf16, tag="Xp3")
                        nc.vector.tensor_tensor(
                            out=Xp3[:].rearrange("p (d h) -> p d h", h=NH),
                            in0=xh_tm[:].rearrange("p (d h) -> p d h", h=NH),
                            in1=d["dt_tm"].rearrange("p (o h) -> p o h", o=1)
                            .to_broadcast((Q, HD, NH)), op=AL.mult)
                        Xpd = zp.tile([128, DI], bf16, tag="Xpd")
                        nc.vector.tensor_tensor(
                            out=Xpd[:].rearrange("p (d h) -> p d h", h=NH),
                            in0=Xp3[:].rearrange("p (d h) -> p d h", h=NH),
                            in1=d["wdec_tm"].rearrange("p (o h) -> p o h", o=1)
                            .to_broadcast((Q, HD, NH)), op=AL.mult)

                        if not first:
                            # CBt[j,i] via PE; multiply into masks (batched)
                            pcb = ptr.tile([128, 128], f32, tag="ps")
                            nc.tensor.matmul(pcb[:], B_fm, C_t[:],
                                             start=True, stop=True)
                            cbt = sp.tile([128, Q], bf16, tag="cbt")
                            nc.vector.tensor_copy(cbt[:], pcb[:])
                            mk = d["mk"]
                            nc.vector.tensor_tensor(
                                out=mk[:, :, 1:Q + 1], in0=mk[:, :, 1:Q + 1],
                                in1=cbt[:].rearrange("p (o i) -> p o i", o=1)
                                .to_broadcast((128, NH, Q)), op=AL.mult)
                            # intra: ypsA[i, h*64:...] = mk_h.T @ Xp3_h
                            #        + D_h * xh_h (identity accumulate)
                            ypsA = pya.tile([128, NH, HD], f32, tag="pya")
                            Xp3v = Xp3[:].rearrange("p (d h) -> p d h", h=NH)
                            xhv = xh_tm[:].rearrange("p (d h) -> p d h", h=NH)
                            for h in range(NH):
                                nc.tensor.matmul(ypsA[:, h, :],
                                                 mk[:, h, 1:Q + 1],
                                                 Xp3v[:, :, h],
                                                 start=True, stop=False)
                                nc.tensor.matmul(
                                    ypsA[:, h, :], identD[:, h, :],
                                    xhv[:, :, h],
                                    start=False, stop=True)
                            # inter: ypsB = C.T @ state
                            ypsB = pyb.tile([128, DI], f32, tag="pyb")
                            for h2 in range(2):
                                nc.tensor.matmul(
                                    ypsB[:, h2 * 512:(h2 + 1) * 512], C_t[:],
                                    state[:, h2 * 512:(h2 + 1) * 512],
                                    start=True, stop=True)
                            # Yt = ypsA + E*ypsB + D*xh ; gate; square+accum
                            Yt = yp.tile([128, DI], bf16, tag=f"Yg{c}")
                            nc.vector.tensor_tensor(
                                out=Yt[:].rearrange("p (d h) -> p d h", h=NH),
                                in0=ypsB[:].rearrange("p (d h) -> p d h", h=NH),
                                in1=d["E_tm"].rearrange("p (o h) -> p o h", o=1)
                                .to_broadcast((128, HD, NH)), op=AL.mult)
                            nc.vector.tensor_tensor(
                                out=Yt[:].rearrange("p (d h) -> p d h", h=NH),
                                in0=Yt[:].rearrange("p (d h) -> p d h", h=NH),
                                in1=ypsA[:].rearrange("p h d -> p d h"),
                                op=AL.add)
                            Ytf = Yt[:]
                            # gate
                            nc.vector.tensor_tensor(out=Ytf, in0=Ytf,
                                                    in1=silu_z[c][:], op=AL.mult)
                            # ss = sum(Yg^2) along features (ACT square+accum)
                            ss = sp.tile([128, 1], f32, tag=f"ss{c}")
                            sqg = zp.tile([128, DI], bf16, tag="Dxh")
                            nc.scalar.activation(sqg[:], Ytf, AF.Square,
                                                 accum_out=ss[:])
                            Yg_list.append(Yt)
                            ss_list.append(ss)

                        # ---- state update ----
                        pdh = pyb.tile([DST, DI], f32, tag="pyb")
                        for h2 in range(2):
                            nc.tensor.matmul(
                                pdh[:, h2 * 512:(h2 + 1) * 512], B_tm[:],
                                Xpd[:, h2 * 512:(h2 + 1) * 512],
                                start=True, stop=True)
                        state_new = st.tile([64, DI], bf16, tag="state")
                        nc.vector.tensor_tensor(
                            out=state_new[:].rearrange("p (d h) -> p d h", h=NH),
                            in0=state[:].rearrange("p (d h) -> p d h", h=NH),
                            in1=d["elast_bc"][:].rearrange("p (o h) -> p o h", o=1)
                            .to_broadcast((DST, HD, NH)), op=AL.mult)
                        nc.vector.tensor_tensor(out=state_new[:], in0=pdh[:],
                                                in1=state_new[:], op=AL.add)
                        state = state_new

                if prev is not None:
                    # ---- deferred MLP of tile it-1 ----
                    ln_fm = prev["ln_fm"]
                    h_fm = op.tile([128, 8, TOK], bf16, tag="h_fm")
                    for mp2 in range(4):
                        ps = pbig.tile([128, 2, TOK], f32, tag="ps")
                        for sub in range(2):
                            mt = mp2 * 2 + sub
                            for k in range(4):
                                nc.tensor.matmul(
                                    ps[:, sub, :],
                                    t_w1[k][:, mt * 128:(mt + 1) * 128],
                                    ln_fm[:, k, :], start=(k == 0), stop=False)
                            nc.tensor.matmul(
                                ps[:, sub, :],
                                t_b1r[:, mt * 128:(mt + 1) * 128],
                                ones_b[:], start=False, stop=True)
                        nc.scalar.activation(
                            h_fm[:, mp2 * 2:mp2 * 2 + 2, :], ps[:], AF.Silu)
                    pt0 = prev["t0"]
                    for mp2 in range(2):
                        ps = pbig.tile([128, 2, TOK], f32, tag="ps")
                        for sub in range(2):
                            mt = mp2 * 2 + sub
                            for k in range(8):
                                nc.tensor.matmul(
                                    ps[:, sub, :],
                                    t_w2[k][:, mt * 128:(mt + 1) * 128],
                                    h_fm[:, k, :], start=(k == 0), stop=False)
                            nc.tensor.matmul(
                                ps[:, sub, :],
                                t_b2r[:, mt * 128:(mt + 1) * 128],
                                ones_b[:], start=False, stop=True)
                        ot = op.tile([128, 2, TOK], bf16, tag="ot")
                        nc.scalar.copy(ot[:], ps[:])
                        for sub in range(2):
                            mt = mp2 * 2 + sub
                            nc.sync.dma_start(
                                out=outT[mt][:, pt0 - HALO:pt0 - HALO + TOK],
                                in_=ot[:, sub, :])

                prev = (None if (last or first)
                        else {"Yg": Yg_list, "ss": ss_list, "t0": t0})

    _BUILT = nc
    return nc


def _host_prep(inputs):
    import ml_dtypes
    bf = ml_dtypes.bfloat16
    x = np.asarray(inputs["x"], np.float32)
    W = np.asarray(inputs["in_proj_w"], np.float32)
    convw = np.asarray(inputs["conv_w"], np.float32)
    convb = np.asarray(inputs["conv_b"], np.float32)
    dtb = np.asarray(inputs["dt_bias"], np.float32)
    A = -np.exp(np.asarray(inputs["A_log"], np.float32).astype(np.float64)).astype(np.float32)
    D = np.asarray(inputs["D"], np.float32)
    rmsw = np.asarray(inputs["rms_w"], np.float32)
    Wout = np.asarray(inputs["out_proj_w"], np.float32)
    lng = np.asarray(inputs["ln_g"], np.float32)
    lnb = np.asarray(inputs["ln_b"], np.float32)
    w1 = np.asarray(inputs["w1"], np.float32)
    b1 = np.asarray(inputs["b1"], np.float32)
    w2 = np.asarray(inputs["w2"], np.float32)
    b2 = np.asarray(inputs["b2"], np.float32)

    w1eff = w1[:, :DM] + w1[:, DM:]
    wout_f = Wout * rmsw[None, :]
    wout_c = wout_f - wout_f.mean(0, keepdims=True)   # centered (folds LN mean)
    # d-major permutation of the 1024 inner features: f' = d*NH + h
    perm = (np.arange(DI) % NH) * HD + np.arange(DI) // NH
    W = np.vstack([W[0:DI][perm],                 # z rows
                   W[DI:DI + DI][perm],           # xh rows of xBC
                   W[DI + DI:]])                  # B/C/dt rows unchanged
    convw = np.vstack([convw[perm], convw[DI:]])
    convb = np.concatenate([convb[perm], convb[DI:]])
    wout_c = wout_c[:, perm]
    w1g = w1eff * lng[None, :]
    b1f = (b1.astype(np.float64) + w1eff.astype(np.float64) @ lnb.astype(np.float64)).astype(np.float32)
    common = {
        "wz": np.ascontiguousarray(W[0:DI].T.reshape(4, 128, DI)).astype(bf),
        "wxbc": np.ascontiguousarray(W[DI:DI + CD].T.reshape(4, 128, CD)).astype(bf),
        "wdt": np.ascontiguousarray(W[DI + CD:].T.reshape(4, 128, NH)).astype(bf),
        "wout": np.ascontiguousarray(wout_c.T.reshape(8, 128, DM)).astype(bf),
        "w1": np.ascontiguousarray(w1g.T.reshape(4, 128, DI)).astype(bf),
        "w2": np.ascontiguousarray(w2.T.reshape(8, 128, DM)).astype(bf),
        "convw": np.ascontiguousarray(convw.reshape(9, 128, 4).transpose(1, 0, 2)).astype(bf),
        "convb": np.ascontiguousarray(convb.reshape(9, 128, 1).transpose(1, 0, 2)),
        "dtb": np.ascontiguousarray(dtb.reshape(NH, 1)),
        "Ah": np.ascontiguousarray(A.reshape(NH, 1)),
        "Drep": np.ascontiguousarray(np.repeat(D, HD).reshape(1, DI)),
        "b1r": np.ascontiguousarray(b1f.reshape(1, DI)).astype(bf),
        "b2r": np.ascontiguousarray(b2.reshape(1, DM)).astype(bf),
    }

    x_rev = x[:, ::-1, :]
    in_maps = []
    for core in range(8):
        b, half = core // 2, core % 2
        if half == 0:
            seg = np.vstack([np.zeros((HALO, DM), np.float32), x_rev[b, :SEG]])
        else:
            seg = x_rev[b, SEG - HALO:2 * SEG]
        m = dict(common)
        m["xT"] = np.ascontiguousarray(seg.T.reshape(4, 128, HALO + SEG)).astype(bf)
        in_maps.append(m)
    return in_maps


_RT = None


def _prepare_runtime(nc, in_maps):
    """Persistent fast-dispatch path (same as baseline kernel)."""
    import jax
    import jax.numpy as jnp
    from jax.sharding import Mesh, PartitionSpec, NamedSharding
    from jax.experimental.shard_map import shard_map
    from concourse import bass2jax, mybir
    bass2jax.install_neuronx_cc_hook()

    n_cores = len(in_maps)
    partition_name = (nc.partition_id_tensor.name
                      if nc.partition_id_tensor else None)
    in_names, out_names, out_avals = [], [], []
    for alloc in nc.m.functions[0].allocations:
        if not isinstance(alloc, mybir.MemoryLocationSet):
            continue
        name = alloc.memorylocations[0].name
        if alloc.kind == "ExternalInput":
            if name != partition_name:
                in_names.append(name)
        elif alloc.kind == "ExternalOutput":
            out_names.append(name)
            out_avals.append(jax.core.ShapedArray(tuple(alloc.tensor_shape),
                                                  mybir.dt.np(alloc.dtype)))
    n_params = len(in_names)
    donate = tuple(range(n_params, n_params + len(out_names)))
    bind_names = list(in_names) + list(out_names)
    if partition_name is not None:
        bind_names.append(partition_name)

    def _body(*args):
        operands = list(args)
        if partition_name is not None:
            operands.append(bass2jax.partition_id_tensor())
        outs = bass2jax._bass_exec_p.bind(
            *operands,
            out_avals=tuple(out_avals),
            in_names=tuple(bind_names),
            out_names=tuple(out_names),
            lowering_input_output_aliases=(),
            sim_require_finite=True,
            sim_require_nnan=True,
            nc=nc,
        )
        return tuple(outs)

    devices = jax.devices()[:n_cores]
    mesh = Mesh(np.asarray(devices), ("core",))
    spec = PartitionSpec("core")
    sharding = NamedSharding(mesh, spec)
    in_specs = (spec,) * (n_params + len(out_names))
    out_specs = (spec,) * len(out_names)
    fn = jax.jit(shard_map(_body, mesh=mesh, in_specs=in_specs,
                           out_specs=out_specs, check_rep=False),
                 donate_argnums=donate, keep_unused=True)

    dev_in = {}
    for name in in_names:
        arrs = [np.asarray(m[name]) for m in in_maps]
        cat = np.concatenate(arrs, axis=0)
        dev_in[name] = jax.device_put(cat, sharding)

    zero_shapes = [(n_cores * a.shape[0], *a.shape[1:]) for a in out_avals]

    def _zeros():
        return [jnp.zeros(s, a.dtype) for s, a in zip(zero_shapes, out_avals)]

    zeros_fn = jax.jit(_zeros, out_shardings=[sharding] * len(out_avals))
    return dict(fn=fn, zeros_fn=zeros_fn, in_names=in_names,
                out_names=out_names, out_avals=out_avals, dev_in=dev_in,
                sharding=sharding, n_cores=n_cores)


def _run(rt, x_cats):
    import jax
    args = []
    for name in rt["in_names"]:
        if name in x_cats:
            args.append(jax.device_put(x_cats[name], rt["sharding"]))
        else:
            args.append(rt["dev_in"][name])
    scratch = rt.pop("_scratch", None)
    if scratch is None:
        scratch = rt["zeros_fn"]()
    outs = rt["fn"](*args, *scratch)
    rt["_scratch"] = outs
    return outs


def _prep_x(inputs):
    import ml_dtypes
    bf = ml_dtypes.bfloat16
    x = np.asarray(inputs["x"], np.float32)
    x_rev = x[:, ::-1, :]
    segs = []
    for core in range(8):
        b, half = core // 2, core % 2
        if half == 0:
            seg = np.vstack([np.zeros((HALO, DM), np.float32), x_rev[b, :SEG]])
        else:
            seg = x_rev[b, SEG - HALO:2 * SEG]
        segs.append(seg.T.reshape(4, 128, HALO + SEG))
    return np.ascontiguousarray(np.concatenate(segs, axis=0)).astype(bf)


_W_KEYS = ("in_proj_w", "conv_w", "conv_b", "dt_bias", "A_log", "D", "rms_w",
           "out_proj_w", "ln_g", "ln_b", "w1", "b1", "w2", "b2")


def kernel(**inputs):
    global _RT
    import jax
    nc = _build()
    fp = tuple(float(np.asarray(inputs[k], np.float64).sum()) for k in _W_KEYS)
    if _RT is None:
        in_maps = _host_prep(inputs)
        _RT = _prepare_runtime(nc, in_maps)
        _RT["_const_key"] = fp
    elif fp != _RT["_const_key"]:
        in_maps = _host_prep(inputs)
        for name in _RT["in_names"]:
            if name == "xT":
                continue
            cat = np.concatenate([np.asarray(m[name]) for m in in_maps], axis=0)
            _RT["dev_in"][name] = jax.device_put(cat, _RT["sharding"])
        _RT["_const_key"] = fp
    xcat = _prep_x(inputs)
    outs = _run(_RT, {"xT": xcat})
    o = np.asarray(outs[_RT["out_names"].index("outT")]).astype(np.float32)
    o = o.reshape(8, 4, 128, SEG)
    x = np.asarray(inputs["x"])
    out_rev = np.zeros((B, L, DM), np.float32)
    for core in range(8):
        b, half = core // 2, core % 2
        out_rev[b, half * SEG:(half + 1) * SEG] = o[core].reshape(DM, SEG).T
    out = np.ascontiguousarray(out_rev[:, ::-1, :])
    return out.astype(x.dtype)


# revision 3
# speedup vs baseline: 1.1961x; 1.1961x over previous
"""Trainium2 Bass kernel for nn_BiMambaBlock — bf16 rewrite.

Strategy (8 NeuronCores, no cross-core communication):
  - Shard (batch=4) x (sequence halves=2) -> 8 cores; 256-token halo on
    second-half cores (SSM state decays to ~0 within the halo).  Host
    reverses time and converts to bf16; kernel runs a forward scan.
  - Per core: 9 tiles of 256 tokens through
    in_proj -> causal conv -> SSD chunked scan (Q=128) -> gated RMSNorm
    -> out_proj -> LayerNorm -> MLP, all in SBUF, all matmuls bf16.
  - Decay masks are built ON-CHIP: dA = exp(dt*A) rows are broadcast
    across partitions with rank-1 PE matmuls, then a DVE scan
    (y_i = y_{i-1}*dA_i + I[j,i]) produces the per-head mask directly
    (upper-triangular by construction).  No DRAM mask traffic.
  - LayerNorm mean is folded into out_proj by centering its columns on
    host; rsqrt is computed as exp(-0.5*ln(v)) so ACT needs only the
    {exp,ln} and {silu} table sets.  ACT work is emitted in two phases
    per tile (exp-set then silu-set) to minimize table reloads.
"""

import numpy as np

# ---- dims ----
DM = 512          # d_model
DST = 64          # d_state
DI = 1024         # d_inner
NH = 16           # heads
HD = 64           # head dim
CD = 1152         # conv dim = DI + 2*DST
B, L = 4, 4096
EPS = 1e-5
HALO, SEG = 256, 2048
TOK = 256         # tokens per pipeline tile
NT = (HALO + SEG) // TOK   # 9
Q = 128           # SSD chunk
NCQ = TOK // Q    # chunks per tile

_BUILT = None
REPEAT = 1


def _patch_concourse(tile_mod, bass_mod):
    """This container's walrus accepts a single sync-wait per instruction.
    Split extra waits onto NoOp / extra Drain instructions."""
    from concourse.vector_clock import ScopedClock
    import json

    def _drain_and_barrier(self, tick_clock, wait_clock):
        nc = self.nc
        drain_inst = nc.sync.drain()
        wait_clock.add_sem_waits(drain_inst.ins,
                                 ScopedClock({None: tick_clock.global_clock}))
        si = drain_inst.ins.sync_info
        waits = list(si.on_wait) if (si is not None and si.on_wait) else []
        if len(waits) > 1:
            si.on_wait = waits[:1]
            name2h = {h.name: h for h in self.sems.allocated().values()}
            for w in waits[1:]:
                d2 = nc.sync.drain()
                d2.wait_op(name2h[w.ant_name], w.wait_value, "sem-ge")
        nc.all_engine_barrier()
        popped = nc._tile_sem_poison_stack.pop()
        assert popped is self._sem_poison
        nc.clear_and_free_semaphores(list(self.sems.allocated().values()))
        nc.all_engine_barrier()

    tile_mod.TileContext._drain_and_barrier = _drain_and_barrier

    def _split_waits(m):
        n = 0
        for f in m.get("functions", []):
            for bb in f.get("blocks", []):
                out = []
                for ins in bb.get("instructions", []):
                    si = ins.get("sync_info")
                    waits = (si or {}).get("on_wait") or []
                    if len(waits) > 1:
                        for i, w in enumerate(waits[:-1]):
                            out.append({
                                "debug": ins.get("debug", 0),
                                "engine": ins["engine"],
                                "ins": [], "outs": [],
                                "name": f"{ins['name']}-ws{i}",
                                "opcode": "NoOp",
                                "sync_info": {"on_update": [], "on_wait": [w]},
                            })
                        si["on_wait"] = waits[-1:]
                        n += 1
                    out.append(ins)
                bb["instructions"] = out
        return n

    if not getattr(bass_mod.Bass, "_wait_split_patched", False):
        orig = bass_mod.Bass.to_json_bytes

        def to_json_bytes(self):
            raw = orig(self)
            m = json.loads(raw)
            if _split_waits(m):
                raw = json.dumps(m).encode()
            return raw

        bass_mod.Bass.to_json_bytes = to_json_bytes
        bass_mod.Bass._wait_split_patched = True


def _build():
    global _BUILT
    if _BUILT is not None:
        return _BUILT
    import concourse.bass as bass
    import concourse.tile as tile
    from concourse import mybir
    from concourse.masks import make_identity
    from contextlib import ExitStack

    _patch_concourse(tile, bass)

    f32 = mybir.dt.float32
    bf16 = mybir.dt.bfloat16
    AL = mybir.AluOpType
    AF = mybir.ActivationFunctionType

    nc = bass.Bass()

    # ---- DRAM I/O (per-core) ----
    # x: bf16, feature-major, (4 k-slices of d_model) x 128 x (HALO+SEG)
    xT = nc.dram_tensor("xT", (4, 128, HALO + SEG), bf16, kind="ExternalInput")
    wz = nc.dram_tensor("wz", (4, 128, DI), bf16, kind="ExternalInput")
    wxbc = nc.dram_tensor("wxbc", (4, 128, CD), bf16, kind="ExternalInput")
    wdt = nc.dram_tensor("wdt", (4, 128, NH), bf16, kind="ExternalInput")
    wout = nc.dram_tensor("wout", (8, 128, DM), bf16, kind="ExternalInput")
    w1 = nc.dram_tensor("w1", (4, 128, DI), bf16, kind="ExternalInput")
    w2 = nc.dram_tensor("w2", (8, 128, DM), bf16, kind="ExternalInput")
    convw = nc.dram_tensor("convw", (128, 9, 4), f32, kind="ExternalInput")
    convb = nc.dram_tensor("convb", (128, 9, 1), f32, kind="ExternalInput")
    dtb = nc.dram_tensor("dtb", (NH, 1), f32, kind="ExternalInput")
    Ah = nc.dram_tensor("Ah", (NH, 1), f32, kind="ExternalInput")
    Drep = nc.dram_tensor("Drep", (1, DI), f32, kind="ExternalInput")
    b1r = nc.dram_tensor("b1r", (1, DI), bf16, kind="ExternalInput")
    b2r = nc.dram_tensor("b2r", (1, DM), bf16, kind="ExternalInput")
    outT = nc.dram_tensor("outT", (4, 128, SEG), bf16, kind="ExternalOutput")

    with tile.TileContext(nc) as tc, ExitStack() as ctx:
        wp = ctx.enter_context(tc.tile_pool(name="wp", bufs=1))
        xp = ctx.enter_context(tc.tile_pool(name="xp", bufs=2))     # x tiles
        cq = ctx.enter_context(tc.tile_pool(name="cq", bufs=2))     # xbc/conv
        sp = ctx.enter_context(tc.tile_pool(name="sp", bufs=2))     # small
        mp = ctx.enter_context(tc.tile_pool(name="mp", bufs=2))     # masks
        zp = ctx.enter_context(tc.tile_pool(name="zp", bufs=2))     # per-chunk big
        yp = ctx.enter_context(tc.tile_pool(name="yp", bufs=2))     # Yg etc
        op = ctx.enter_context(tc.tile_pool(name="op", bufs=2))     # out stage
        st = ctx.enter_context(tc.tile_pool(name="st", bufs=2))     # state/stash
        dp = ctx.enter_context(tc.tile_pool(name="dp", bufs=2, space="DRAM"))
        pbig = ctx.enter_context(tc.tile_pool(name="pbig", bufs=2, space="PSUM"))
        ptr = ctx.enter_context(tc.tile_pool(name="ptr", bufs=2, space="PSUM"))
        pya = ctx.enter_context(tc.tile_pool(name="pya", bufs=1, space="PSUM"))
        pyb = ctx.enter_context(tc.tile_pool(name="pyb", bufs=1, space="PSUM"))

        # ---- load weights / constants ----
        def ld(name, dram, shape, dt=bf16):
            t = wp.tile(list(shape), dt, tag=name)
            nc.sync.dma_start(out=t[:], in_=dram[:])
            return t

        t_wz = [ld(f"wz{k}", wz[k], (128, DI)) for k in range(4)]
        t_wxbc = [ld(f"wxbc{k}", wxbc[k], (128, CD)) for k in range(4)]
        t_wdt = [ld(f"wdt{k}", wdt[k], (128, NH)) for k in range(4)]
        t_wout = [ld(f"wout{k}", wout[k], (128, DM)) for k in range(8)]
        t_w1 = [ld(f"w1{k}", w1[k], (128, DI)) for k in range(4)]
        t_w2 = [ld(f"w2{k}", w2[k], (128, DM)) for k in range(8)]
        t_convw = ld("convw", convw, (128, 9, 4), f32)
        t_convb = ld("convb", convb, (128, 9, 1), f32)
        t_dtb = ld("dtb", dtb, (NH, 1), f32)
        t_A = ld("Ah", Ah, (NH, 1), f32)
        t_Dbc = wp.tile([128, DI], f32, tag="Dbc")
        nc.sync.dma_start(out=t_Dbc[:], in_=Drep[:].to_broadcast((128, DI)))

        identb = wp.tile([128, 128], bf16, tag="identb")
        make_identity(nc, identb[:])
        identf = wp.tile([128, 128], f32, tag="identf")
        make_identity(nc, identf[:])
        ones_b = wp.tile([1, TOK], bf16, tag="ones_b")
        nc.vector.memset(ones_b[:], 1.0)
        t_b1r = wp.tile([1, DI], bf16, tag="b1r")
        nc.sync.dma_start(out=t_b1r[:], in_=b1r[:])
        t_b2r = wp.tile([1, DM], bf16, tag="b2r")
        nc.sync.dma_start(out=t_b2r[:], in_=b2r[:])
        epsc = wp.tile([128, 1], f32, tag="epsc")
        nc.vector.memset(epsc[:], EPS)
        zrow = wp.tile([NH, 1], f32, tag="zrow")
        nc.vector.memset(zrow[:], 0.0)
        ones_col = wp.tile([128, 1], bf16, tag="ones_col")
        nc.vector.memset(ones_col[:], 1.0)
        # shifted-identity injection pattern for 8-head batched mask scans
        identx = wp.tile([128, 8 * (Q + 1)], bf16, tag="identx")
        nc.vector.memset(identx[:], 0.0)
        for g4 in range(8):
            nc.vector.tensor_copy(
                identx[:, g4 * (Q + 1) + 1:(g4 + 1) * (Q + 1)], identb[:])
        # persistent DRAM scratch for dA partition-broadcast (129-pitch,
        # separator column zeroed once) and for elast rows
        zsep = wp.tile([NH, Q + 1], bf16, tag="zsep")
        nc.vector.memset(zsep[:], 0.0)
        dAd = dp.tile([2, 2, NH * (Q + 1)], bf16, tag="dAd")
        for t2 in range(2):
            for c2 in range(2):
                nc.sync.dma_start(
                    out=dAd[t2][c2:c2 + 1, :].rearrange(
                        "o (h i) -> (o h) i", h=NH), in_=zsep[:])
        eld = dp.tile([2, 2, NH], bf16, tag="eld")

        # D-scaled identities: ypsA[:,h,:] += (D_h I) @ xh_h  (skip term)
        identD = wp.tile([128, NH, 128], bf16, tag="identD")
        for h4 in range(NH):
            nc.vector.tensor_scalar(identD[:, h4, :], identb[:],
                                    t_Dbc[:, h4 * HD:h4 * HD + 1], None,
                                    op0=AL.mult)
        # persistent state / conv stash
        state = st.tile([64, DI], bf16, tag="state")
        nc.vector.memset(state[:], 0.0)
        stash = st.tile([128, 9, 3], bf16, tag="stash")
        nc.vector.memset(stash[:], 0.0)

        # cross-tile staging (tile it computes Yg/ss; out_proj..MLP of
        # tile it runs during iteration it+1, after rstd in exp-phase)
        prev = None   # dict with Yg[c], ss[c] from previous tile

        for _rep in range(REPEAT):
            for it in range(NT + 1):
                first, last = (it == 0), (it == NT)
                t0 = it * TOK

                # =========================================================
                # PHASE A (exp/ln table set on ACT):
                #   dt path for tile `it` + rstd/out_proj/LN for tile it-1
                # =========================================================
                if not last:
                    # ---- x tile (feature-major bf16) ----
                    x_fm = xp.tile([128, 4, TOK], bf16, tag="x_fm")
                    for k in range(4):
                        nc.sync.dma_start(out=x_fm[:, k, :],
                                          in_=xT[k][:, t0:t0 + TOK])

                    # ---- dt path: psd = wdt.T @ x ----
                    psd = pbig.tile([NH, TOK], f32, tag="ps")
                    for k in range(4):
                        nc.tensor.matmul(psd[:], t_wdt[k][:], x_fm[:, k, :],
                                         start=(k == 0), stop=(k == 3))
                    # softplus(x+b) = ln(1 + exp(x+b))
                    dt_e = sp.tile([NH, TOK], f32, tag="dt_e")
                    nc.scalar.activation(dt_e[:], psd[:], AF.Exp,
                                         bias=t_dtb[:], scale=1.0)
                    nc.vector.tensor_scalar_add(dt_e[:], dt_e[:], 1.0)
                    dt_fm = sp.tile([NH, TOK], f32, tag="dt_fm")
                    nc.scalar.activation(dt_fm[:], dt_e[:], AF.Ln)
                    dtA_fm = sp.tile([NH, TOK], f32, tag="dtA_fm")
                    nc.vector.tensor_scalar_mul(dtA_fm[:], dt_fm[:], t_A[:])
                    dtb16 = sp.tile([NH, TOK], bf16, tag="dtb16")
                    nc.scalar.copy(dtb16[:], dt_fm[:])
                    # dA = exp(dtA)  (bf16 factors for the mask scans)
                    dA_fm = sp.tile([NH, TOK], bf16, tag="dA_fm")
                    nc.scalar.activation(dA_fm[:], dtA_fm[:], AF.Exp)
                    t2 = it % 2

                    # per-chunk dt-derived quantities
                    ch = []
                    for c in range(NCQ):
                        csl = slice(c * Q, (c + 1) * Q)
                        d = {}
                        s_fm = sp.tile([NH, Q], f32, tag=f"s_fm{c}")
                        nc.vector.tensor_tensor_scan(
                            s_fm[:], dtA_fm[:, csl],
                            zrow[:].to_broadcast((NH, Q)),
                            0.0, op0=AL.add, op1=AL.add)
                        d["s_fm"] = s_fm
                        # wdec_fm = exp(s_last - s)  (per-partition scalar)
                        wde = sp.tile([NH, Q], f32, tag=f"wde{c}")
                        nc.vector.tensor_scalar(
                            wde[:], s_fm[:], s_fm[:, Q - 1:Q], None,
                            op0=AL.subtract)
                        wdec_fm = sp.tile([NH, Q], bf16, tag=f"wdecf{c}")
                        nc.scalar.activation(wdec_fm[:], wde[:], AF.Exp,
                                             scale=-1.0)
                        # E_fm = exp(s)
                        E_fm = sp.tile([NH, Q], bf16, tag=f"Ef{c}")
                        nc.scalar.activation(E_fm[:], s_fm[:], AF.Exp)
                        # elast column = exp(s_last) (bf16 [NH,1])
                        ecol = sp.tile([NH, 1], bf16, tag=f"ecol{c}")
                        nc.scalar.activation(ecol[:], s_fm[:, Q - 1:Q], AF.Exp)
                        # transposes (token-major): dt, wdec, E
                        ptx = ptr.tile([Q, 3, NH], bf16, tag="ps")
                        nc.tensor.transpose(ptx[:, 0, :], dtb16[:, csl],
                                            identb[0:NH, 0:NH])
                        nc.tensor.transpose(ptx[:, 1, :], wdec_fm[:],
                                            identb[0:NH, 0:NH])
                        nc.tensor.transpose(ptx[:, 2, :], E_fm[:],
                                            identb[0:NH, 0:NH])
                        tws = sp.tile([Q, 3, NH], bf16, tag=f"tws{c}")
                        nc.vector.tensor_copy(tws[:], ptx[:])
                        dt_tm = tws[:, 0, :]
                        wdec_tm = tws[:, 1, :]
                        E_tm = tws[:, 2, :]
                        # elast broadcast via DRAM bounce [64, 16]
                        nc.sync.dma_start(out=eld[t2][c:c + 1, :]
                                          .rearrange("o h -> (o h)"), in_=ecol[:])
                        elast_bc = sp.tile([DST, NH], bf16, tag="elbc")
                        nc.sync.dma_start(
                            out=elast_bc[:],
                            in_=eld[t2][c:c + 1, :].to_broadcast((DST, NH)))
                        d["dt_tm"], d["wdec_tm"], d["E_tm"] = dt_tm, wdec_tm, E_tm
                        d["elast_bc"] = elast_bc
                        # masks: DRAM partition-broadcast of dA, then 4
                        # batched 4-head scans (product recurrence)
                        if not first:
                            nc.sync.dma_start(
                                out=dAd[t2][c:c + 1, :].rearrange(
                                    "o (h i) -> (o h) i", h=NH)[:, 1:Q + 1],
                                in_=dA_fm[:, csl])
                            dA_bc = mp.tile([128, NH * (Q + 1)], bf16, tag="dabc")
                            nc.sync.dma_start(
                                out=dA_bc[:],
                                in_=dAd[t2][c:c + 1, :].to_broadcast(
                                    (128, NH * (Q + 1))))
                            mk = mp.tile([128, NH, Q + 1], bf16, tag="mk")
                            mkf = mk[:].rearrange("p h i -> p (h i)")
                            for g in range(2):
                                gsl = slice(g * 8 * (Q + 1), (g + 1) * 8 * (Q + 1))
                                nc.vector.tensor_tensor_scan(
                                    mkf[:, gsl], dA_bc[:, gsl], identx[:],
                                    0.0, op0=AL.mult, op1=AL.add)
                            d["mk"] = mk
                        ch.append(d)

                if prev is not None:
                    # ---- deferred tail of tile it-1 (exp/ln set) ----
                    # RMS rstd per chunk: rstd = exp(-0.5*ln(ms))
                    yn_fm = yp.tile([128, 8, TOK], bf16, tag="yn_fm")
                    for c in range(NCQ):
                        csl = slice(c * Q, (c + 1) * Q)
                        ssl = prev["ss"][c]
                        lnm = sp.tile([Q, 1], f32, tag="lnm")
                        nc.scalar.activation(lnm[:], ssl[:], AF.Ln,
                                             bias=epsc[0:Q, :], scale=1.0 / DI)
                        rstd = sp.tile([Q, 1], f32, tag="rstd")
                        nc.scalar.activation(rstd[:], lnm[:], AF.Exp, scale=-0.5)
                        yn = yp.tile([Q, DI], bf16, tag="yn")
                        nc.vector.tensor_scalar_mul(yn[:], prev["Yg"][c][:],
                                                    rstd[:])
                        # transpose yn -> feature-major (batched drains)
                        for g4 in range(2):
                            ptn = ptr.tile([128, 4, 128], bf16, tag="ps")
                            for mm in range(4):
                                nc.tensor.transpose(
                                    ptn[:, mm, :],
                                    yn[:, (g4 * 4 + mm) * 128:(g4 * 4 + mm + 1) * 128],
                                    identb[:])
                            nc.vector.tensor_copy(
                                yn_fm[:, g4 * 4:(g4 + 1) * 4, csl], ptn[:])
                    # out_proj (centered weights) -> ym psum [128, 4, TOK]
                    pym = pya.tile([128, 4, TOK], f32, tag="pya")
                    for mt in range(4):
                        for k in range(8):
                            nc.tensor.matmul(
                                pym[:, mt, :],
                                t_wout[k][:, mt * 128:(mt + 1) * 128],
                                yn_fm[:, k, :], start=(k == 0), stop=(k == 7))
                    # var = mean(ym^2): square on ACT (any set), PE reduce
                    sq4 = op.tile([128, 4, TOK], bf16, tag="sq4")
                    nc.scalar.activation(sq4[:], pym[:], AF.Square)
                    pv = ptr.tile([1, TOK], f32, tag="ps")
                    for k in range(4):
                        nc.tensor.matmul(pv[:], ones_col[:], sq4[:, k, :],
                                         start=(k == 0), stop=(k == 3))
                    lnv = sp.tile([1, TOK], f32, tag="lnv")
                    nc.scalar.activation(lnv[:], pv[:], AF.Ln,
                                         bias=epsc[0:1, :], scale=1.0 / DM)
                    rstdr = sp.tile([1, TOK], bf16, tag="rstdr")
                    nc.scalar.activation(rstdr[:], lnv[:], AF.Exp, scale=-0.5)
                    # broadcast rstd row across partitions (rank-1)
                    prs = ptr.tile([128, TOK], f32, tag="ps")
                    nc.tensor.matmul(prs[:], ones_b[:, 0:128], rstdr[:],
                                     start=True, stop=True)
                    rstd_bc = op.tile([128, TOK], bf16, tag="rstd_bc")
                    nc.vector.tensor_copy(rstd_bc[:], prs[:])
                    # ln = ym * rstd_bc  (mean already removed by centering)
                    ln_fm = op.tile([128, 4, TOK], bf16, tag="ln_fm")
                    nc.vector.tensor_tensor(
                        out=ln_fm[:], in0=pym[:],
                        in1=rstd_bc[:].rearrange("p (o t) -> p o t", o=1)
                        .to_broadcast((128, 4, TOK)), op=AL.mult)
                    prev["ln_fm"] = ln_fm

                # =========================================================
                # PHASE B (silu table set on ACT):
                #   conv/SSD/gate for tile `it` + MLP of tile it-1
                # =========================================================
                if not last:
                    # ---- in_proj xBC (feature-major) ----
                    xbc_ext = cq.tile([128, 9, 3 + TOK], bf16, tag="xbc_ext")
                    nc.scalar.copy(xbc_ext[:, :, 0:3], stash[:])
                    conv = cq.tile([128, 9, TOK], bf16, tag="conv")
                    for mp2 in range(5):
                        nmt = 2 if mp2 < 4 else 1
                        ps = pbig.tile([128, 2, TOK], f32, tag="ps")
                        for sub in range(nmt):
                            mt = mp2 * 2 + sub
                            for k in range(4):
                                nc.tensor.matmul(
                                    ps[:, sub, :],
                                    t_wxbc[k][:, mt * 128:(mt + 1) * 128],
                                    x_fm[:, k, :], start=(k == 0), stop=(k == 3))
                        nc.scalar.copy(
                            xbc_ext[:, mp2 * 2:mp2 * 2 + nmt, 3:3 + TOK],
                            ps[:, 0:nmt, :])
                        for sub in range(nmt):
                            mt = mp2 * 2 + sub
                            nc.scalar.activation(
                                conv[:, mt, :], ps[:, sub, :], AF.Identity,
                                scale=t_convw[:, mt, 3:4],
                                bias=t_convb[:, mt, :])
                    stash_new = st.tile([128, 9, 3], bf16, tag="stash")
                    nc.scalar.copy(stash_new[:], xbc_ext[:, :, TOK:TOK + 3])
                    stash = stash_new

                    # ---- conv (no silu yet) ----
                    for mt in range(9):
                        for k in (2, 1, 0):
                            nc.vector.scalar_tensor_tensor(
                                conv[:, mt, :], xbc_ext[:, mt, k:k + TOK],
                                t_convw[:, mt, k:k + 1],
                                conv[:, mt, :], op0=AL.mult, op1=AL.add)
                    # B/C rows: silu in feature-major
                    bc_fm = cq.tile([128, TOK], bf16, tag="bc_fm")
                    nc.scalar.activation(bc_fm[:], conv[:, 8, :], AF.Silu)

                    # ---- z (token-major, silu) ----
                    silu_z = [None, None]
                    if not first:
                        for c in range(NCQ):
                            csl = slice(c * 128, (c + 1) * 128)
                            zt = zp.tile([128, DI], bf16, tag=f"silu_z{c}")
                            for h2 in range(2):
                                ps = pbig.tile([128, 512], f32, tag="ps")
                                for k in range(4):
                                    nc.tensor.matmul(
                                        ps[:], x_fm[:, k, csl],
                                        t_wz[k][:, h2 * 512:(h2 + 1) * 512],
                                        start=(k == 0), stop=(k == 3))
                                nc.scalar.activation(
                                    zt[:, h2 * 512:(h2 + 1) * 512], ps[:], AF.Silu)
                            silu_z[c] = zt

                    # ---- per-chunk SSD ----
                    Yg_list, ss_list = [], []
                    for c in range(NCQ):
                        csl = slice(c * 128, (c + 1) * 128)
                        d = ch[c]
                        # xh: transpose (4 per psum bank) + batched silu
                        xh_tm = zp.tile([128, DI], bf16, tag="xh_tm")
                        for g4 in range(2):
                            pt = ptr.tile([128, 4, 128], bf16, tag="ps")
                            for mm in range(4):
                                nc.tensor.transpose(
                                    pt[:, mm, :],
                                    conv[:, g4 * 4 + mm, csl], identb[:])
                            nc.scalar.activation(
                                xh_tm[:, g4 * 512:(g4 + 1) * 512],
                                pt[:].rearrange("p m t -> p (m t)"), AF.Silu)
                        # B_fm / C: B rows 0-63 of bc_fm, C rows 64-127 -> move C
                        B_fm = bc_fm[0:DST, csl]
                        C_t = sp.tile([DST, Q], bf16, tag=f"C_t{c}")
                        nc.sync.dma_start(out=C_t[:], in_=bc_fm[DST:128, csl])
                        # B token-major (for state update lhsT)
                        pbt = ptr.tile([Q, DST], bf16, tag="ps")
                        nc.tensor.transpose(pbt[:], B_fm, identb[0:DST, 0:DST])
                        B_tm = sp.tile([Q, DST], bf16, tag=f"B_tm{c}")
                        nc.vector.tensor_copy(B_tm[:], pbt[:])
                        # Xp3 = dt * xh ; Xpd = Xp3 * wdec
                        Xp3 = zp.tile([128, DI], bf16, tag="Xp3")
                        nc.vector.tensor_tensor(
                            out=Xp3[:].rearrange("p (d h) -> p d h", h=NH),
                            in0=xh_tm[:].rearrange("p (d h) -> p d h", h=NH),
                            in1=d["dt_tm"].rearrange("p (o h) -> p o h", o=1)
                            .to_broadcast((Q, HD, NH)), op=AL.mult)
                        Xpd = zp.tile([128, DI], bf16, tag="Xpd")
                        nc.vector.tensor_tensor(
                            out=Xpd[:].rearrange("p (d h) -> p d h", h=NH),
                            in0=Xp3[:].rearrange("p (d h) -> p d h", h=NH),
                            in1=d["wdec_tm"].rearrange("p (o h) -> p o h", o=1)
                            .to_broadcast((Q, HD, NH)), op=AL.mult)

                        if not first:
                            # CBt[j,i] via PE; multiply into masks (batched)
                            pcb = ptr.tile([128, 128], f32, tag="ps")
                            nc.tensor.matmul(pcb[:], B_fm, C_t[:],
                                             start=True, stop=True)
                            cbt = sp.tile([128, Q], bf16, tag="cbt")
                            nc.scalar.copy(cbt[:], pcb[:])
                            mk = d["mk"]
                            nc.vector.tensor_tensor(
                                out=mk[:, :, 1:Q + 1], in0=mk[:, :, 1:Q + 1],
                                in1=cbt[:].rearrange("p (o i) -> p o i", o=1)
                                .to_broadcast((128, NH, Q)), op=AL.mult)
                            # intra: ypsA[i, h*64:...] = mk_h.T @ Xp3_h
                            #        + D_h * xh_h (identity accumulate)
                            ypsA = pya.tile([128, NH, HD], f32, tag="pya")
                            Xp3v = Xp3[:].rearrange("p (d h) -> p d h", h=NH)
                            xhv = xh_tm[:].rearrange("p (d h) -> p d h", h=NH)
                            for h in range(NH):
                                nc.tensor.matmul(ypsA[:, h, :],
                                                 mk[:, h, 1:Q + 1],
                                                 Xp3v[:, :, h],
                                                 start=True, stop=False)
                                nc.tensor.matmul(
                                    ypsA[:, h, :], identD[:, h, :],
                                    xhv[:, :, h],
                                    start=False, stop=True)
                            # inter: ypsB = C.T @ state
                            ypsB = pyb.tile([128, DI], f32, tag="pyb")
                            for h2 in range(2):
                                nc.tensor.matmul(
                                    ypsB[:, h2 * 512:(h2 + 1) * 512], C_t[:],
                                    state[:, h2 * 512:(h2 + 1) * 512],
                                    start=True, stop=True)
                            # Yt = ypsA + E*ypsB + D*xh ; gate; square+accum
                            Yt = yp.tile([128, DI], bf16, tag=f"Yg{c}")
                            nc.vector.tensor_tensor(
                                out=Yt[:].rearrange("p (d h) -> p d h", h=NH),
                                in0=ypsB[:].rearrange("p (d h) -> p d h", h=NH),
                                in1=d["E_tm"].rearrange("p (o h) -> p o h", o=1)
                                .to_broadcast((128, HD, NH)), op=AL.mult)
                            nc.vector.tensor_tensor(
                                out=Yt[:].rearrange("p (d h) -> p d h", h=NH),
                                in0=Yt[:].rearrange("p (d h) -> p d h", h=NH),
                                in1=ypsA[:].rearrange("p h d -> p d h"),
                                op=AL.add)
                            Ytf = Yt[:]
                            # gate
                            nc.vector.tensor_tensor(out=Ytf, in0=Ytf,
                                                    in1=silu_z[c][:], op=AL.mult)
                            # ss = sum(Yg^2) along features (ACT square+accum)
                            ss = sp.tile([128, 1], f32, tag=f"ss{c}")
                            sqg = zp.tile([128, DI], bf16, tag="Dxh")
                            nc.scalar.activation(sqg[:], Ytf, AF.Square,
                                                 accum_out=ss[:])
                            Yg_list.append(Yt)
                            ss_list.append(ss)

                        # ---- state update ----
                        pdh = pyb.tile([DST, DI], f32, tag="pyb")
                        for h2 in range(2):
                            nc.tensor.matmul(
                                pdh[:, h2 * 512:(h2 + 1) * 512], B_tm[:],
                                Xpd[:, h2 * 512:(h2 + 1) * 512],
                                start=True, stop=True)
                        state_new = st.tile([64, DI], bf16, tag="state")
                        nc.vector.tensor_tensor(
                            out=state_new[:].rearrange("p (d h) -> p d h", h=NH),
                            in0=state[:].rearrange("p (d h) -> p d h", h=NH),
                            in1=d["elast_bc"][:].rearrange("p (o h) -> p o h", o=1)
                            .to_broadcast((DST, HD, NH)), op=AL.mult)
                        nc.vector.tensor_tensor(out=state_new[:], in0=pdh[:],
                                                in1=state_new[:], op=AL.add)
                        state = state_new

                if prev is not None:
                    # ---- deferred MLP of tile it-1 ----
                    ln_fm = prev["ln_fm"]
                    h_fm = op.tile([128, 8, TOK], bf16, tag="h_fm")
                    for mp2 in range(4):
                        ps = pbig.tile([128, 2, TOK], f32, tag="ps")
                        for sub in range(2):
                            mt = mp2 * 2 + sub
                            for k in range(4):
                                nc.tensor.matmul(
                                    ps[:, sub, :],
                                    t_w1[k][:, mt * 128:(mt + 1) * 128],
                                    ln_fm[:, k, :], start=(k == 0), stop=False)
                            nc.tensor.matmul(
                                ps[:, sub, :],
                                t_b1r[:, mt * 128:(mt + 1) * 128],
                                ones_b[:], start=False, stop=True)
                        nc.scalar.activation(
                            h_fm[:, mp2 * 2:mp2 * 2 + 2, :], ps[:], AF.Silu)
                    pt0 = prev["t0"]
                    for mp2 in range(2):
                        ps = pbig.tile([128, 2, TOK], f32, tag="ps")
                        for sub in range(2):
                            mt = mp2 * 2 + sub
                            for k in range(8):
                                nc.tensor.matmul(
                                    ps[:, sub, :],
                                    t_w2[k][:, mt * 128:(mt + 1) * 128],
                                    h_fm[:, k, :], start=(k == 0), stop=False)
                            nc.tensor.matmul(
                                ps[:, sub, :],
                                t_b2r[:, mt * 128:(mt + 1) * 128],
                                ones_b[:], start=False, stop=True)
                        ot = op.tile([128, 2, TOK], bf16, tag="ot")
                        nc.scalar.copy(ot[:], ps[:])
                        for sub in range(2):
                            mt = mp2 * 2 + sub
                            nc.sync.dma_start(
                                out=outT[mt][:, pt0 - HALO:pt0 - HALO + TOK],
                                in_=ot[:, sub, :])

                prev = (None if (last or first)
                        else {"Yg": Yg_list, "ss": ss_list, "t0": t0})

    _BUILT = nc
    return nc


def _host_prep(inputs):
    import ml_dtypes
    bf = ml_dtypes.bfloat16
    x = np.asarray(inputs["x"], np.float32)
    W = np.asarray(inputs["in_proj_w"], np.float32)
    convw = np.asarray(inputs["conv_w"], np.float32)
    convb = np.asarray(inputs["conv_b"], np.float32)
    dtb = np.asarray(inputs["dt_bias"], np.float32)
    A = -np.exp(np.asarray(inputs["A_log"], np.float32).astype(np.float64)).astype(np.float32)
    D = np.asarray(inputs["D"], np.float32)
    rmsw = np.asarray(inputs["rms_w"], np.float32)
    Wout = np.asarray(inputs["out_proj_w"], np.float32)
    lng = np.asarray(inputs["ln_g"], np.float32)
    lnb = np.asarray(inputs["ln_b"], np.float32)
    w1 = np.asarray(inputs["w1"], np.float32)
    b1 = np.asarray(inputs["b1"], np.float32)
    w2 = np.asarray(inputs["w2"], np.float32)
    b2 = np.asarray(inputs["b2"], np.float32)

    w1eff = w1[:, :DM] + w1[:, DM:]
    wout_f = Wout * rmsw[None, :]
    wout_c = wout_f - wout_f.mean(0, keepdims=True)   # centered (folds LN mean)
    # d-major permutation of the 1024 inner features: f' = d*NH + h
    perm = (np.arange(DI) % NH) * HD + np.arange(DI) // NH
    W = np.vstack([W[0:DI][perm],                 # z rows
                   W[DI:DI + DI][perm],           # xh rows of xBC
                   W[DI + DI:]])                  # B/C/dt rows unchanged
    convw = np.vstack([convw[perm], convw[DI:]])
    convb = np.concatenate([convb[perm], convb[DI:]])
    wout_c = wout_c[:, perm]
    w1g = w1eff * lng[None, :]
    b1f = (b1.astype(np.float64) + w1eff.astype(np.float64) @ lnb.astype(np.float64)).astype(np.float32)
    common = {
        "wz": np.ascontiguousarray(W[0:DI].T.reshape(4, 128, DI)).astype(bf),
        "wxbc": np.ascontiguousarray(W[DI:DI + CD].T.reshape(4, 128, CD)).astype(bf),
        "wdt": np.ascontiguousarray(W[DI + CD:].T.reshape(4, 128, NH)).astype(bf),
        "wout": np.ascontiguousarray(wout_c.T.reshape(8, 128, DM)).astype(bf),
        "w1": np.ascontiguousarray(w1g.T.reshape(4, 128, DI)).astype(bf),
        "w2": np.ascontiguousarray(w2.T.reshape(8, 128, DM)).astype(bf),
        "convw": np.ascontiguousarray(convw.reshape(9, 128, 4).transpose(1, 0, 2)),
        "convb": np.ascontiguousarray(convb.reshape(9, 128, 1).transpose(1, 0, 2)),
        "dtb": np.ascontiguousarray(dtb.reshape(NH, 1)),
        "Ah": np.ascontiguousarray(A.reshape(NH, 1)),
        "Drep": np.ascontiguousarray(np.repeat(D, HD).reshape(1, DI)),
        "b1r": np.ascontiguousarray(b1f.reshape(1, DI)).astype(bf),
        "b2r": np.ascontiguousarray(b2.reshape(1, DM)).astype(bf),
    }

    x_rev = x[:, ::-1, :]
    in_maps = []
    for core in range(8):
        b, half = core // 2, core % 2
        if half == 0:
            seg = np.vstack([np.zeros((HALO, DM), np.float32), x_rev[b, :SEG]])
        else:
            seg = x_rev[b, SEG - HALO:2 * SEG]
        m = dict(common)
        m["xT"] = np.ascontiguousarray(seg.T.reshape(4, 128, HALO + SEG)).astype(bf)
        in_maps.append(m)
    return in_maps


_RT = None


def _prepare_runtime(nc, in_maps):
    """Persistent fast-dispatch path (same as baseline kernel)."""
    import jax
    import jax.numpy as jnp
    from jax.sharding import Mesh, PartitionSpec, NamedSharding
    from jax.experimental.shard_map import shard_map
    from concourse import bass2jax, mybir
    bass2jax.install_neuronx_cc_hook()

    n_cores = len(in_maps)
    partition_name = (nc.partition_id_tensor.name
                      if nc.partition_id_tensor else None)
    in_names, out_names, out_avals = [], [], []
    for alloc in nc.m.functions[0].allocations:
        if not isinstance(alloc, mybir.MemoryLocationSet):
            continue
        name = alloc.memorylocations[0].name
        if alloc.kind == "ExternalInput":
            if name != partition_name:
                in_names.append(name)
        elif alloc.kind == "ExternalOutput":
            out_names.append(name)
            out_avals.append(jax.core.ShapedArray(tuple(alloc.tensor_shape),
                                                  mybir.dt.np(alloc.dtype)))
    n_params = len(in_names)
    donate = tuple(range(n_params, n_params + len(out_names)))
    bind_names = list(in_names) + list(out_names)
    if partition_name is not None:
        bind_names.append(partition_name)

    def _body(*args):
        operands = list(args)
        if partition_name is not None:
            operands.append(bass2jax.partition_id_tensor())
        outs = bass2jax._bass_exec_p.bind(
            *operands,
            out_avals=tuple(out_avals),
            in_names=tuple(bind_names),
            out_names=tuple(out_names),
            lowering_input_output_aliases=(),
            sim_require_finite=True,
            sim_require_nnan=True,
            nc=nc,
        )
        return tuple(outs)

    devices = jax.devices()[:n_cores]
    mesh = Mesh(np.asarray(devices), ("core",))
    spec = PartitionSpec("core")
    sharding = NamedSharding(mesh, spec)
    in_specs = (spec,) * (n_params + len(out_names))
    out_specs = (spec,) * len(out_names)
    fn = jax.jit(shard_map(_body, mesh=mesh, in_specs=in_specs,
                           out_specs=out_specs, check_rep=False),
                 donate_argnums=donate, keep_unused=True)

    dev_in = {}
    for name in in_names:
        arrs = [np.asarray(m[name]) for m in in_maps]
        cat = np.concatenate(arrs, axis=0)
        dev_in[name] = jax.device_put(cat, sharding)

    zero_shapes = [(n_cores * a.shape[0], *a.shape[1:]) for a in out_avals]

    def _zeros():
        return [jnp.zeros(s, a.dtype) for s, a in zip(zero_shapes, out_avals)]

    zeros_fn = jax.jit(_zeros, out_shardings=[sharding] * len(out_avals))
    return dict(fn=fn, zeros_fn=zeros_fn, in_names=in_names,
                out_names=out_names, out_avals=out_avals, dev_in=dev_in,
                sharding=sharding, n_cores=n_cores)


def _run(rt, x_cats):
    import jax
    args = []
    for name in rt["in_names"]:
        if name in x_cats:
            args.append(jax.device_put(x_cats[name], rt["sharding"]))
        else:
            args.append(rt["dev_in"][name])
    scratch = rt.pop("_scratch", None)
    if scratch is None:
        scratch = rt["zeros_fn"]()
    outs = rt["fn"](*args, *scratch)
    rt["_scratch"] = outs
    return outs


def _prep_x(inputs):
    import ml_dtypes
    bf = ml_dtypes.bfloat16
    x = np.asarray(inputs["x"], np.float32)
    x_rev = x[:, ::-1, :]
    segs = []
    for core in range(8):
        b, half = core // 2, core % 2
        if half == 0:
            seg = np.vstack([np.zeros((HALO, DM), np.float32), x_rev[b, :SEG]])
        else:
            seg = x_rev[b, SEG - HALO:2 * SEG]
        segs.append(seg.T.reshape(4, 128, HALO + SEG))
    return np.ascontiguousarray(np.concatenate(segs, axis=0)).astype(bf)


_W_KEYS = ("in_proj_w", "conv_w", "conv_b", "dt_bias", "A_log", "D", "rms_w",
           "out_proj_w", "ln_g", "ln_b", "w1", "b1", "w2", "b2")


def kernel(**inputs):
    global _RT
    import jax
    nc = _build()
    fp = tuple(float(np.asarray(inputs[k], np.float64).sum()) for k in _W_KEYS)
    if _RT is None:
        in_maps = _host_prep(inputs)
        _RT = _prepare_runtime(nc, in_maps)
        _RT["_const_key"] = fp
    elif fp != _RT["_const_key"]:
        in_maps = _host_prep(inputs)
        for name in _RT["in_names"]:
            if name == "xT":
                continue
            cat = np.concatenate([np.asarray(m[name]) for m in in_maps], axis=0)
            _RT["dev_in"][name] = jax.device_put(cat, _RT["sharding"])
        _RT["_const_key"] = fp
    xcat = _prep_x(inputs)
    outs = _run(_RT, {"xT": xcat})
    o = np.asarray(outs[_RT["out_names"].index("outT")]).astype(np.float32)
    o = o.reshape(8, 4, 128, SEG)
    x = np.asarray(inputs["x"])
    out_rev = np.zeros((B, L, DM), np.float32)
    for core in range(8):
        b, half = core // 2, core % 2
        out_rev[b, half * SEG:(half + 1) * SEG] = o[core].reshape(DM, SEG).T
    out = np.ascontiguousarray(out_rev[:, ::-1, :])
    return out.astype(x.dtype)
